# revision 1
# baseline (speedup 1.0000x reference)
"""Trainium2 Bass kernel for nn_GroupPointEncoder.

Reference computation (G=4, B=8, N=2048, F=128):
  std = 2 or 4 per point by label class
  coords = [point_coord, (point_coord + noise*std)[1:]]           # [G,B,N,3]
  normed = (coords - low) / (high - low)
  pe     = interleaved sin/cos embedding, (y,x,z) order            # [G,B,N,384]
  h      = relu(pe @ W1.T + b1)                                    # [G,B,N,512]
  pos    = h @ W2.T + b2                                           # [G,B,N,256]
  query  = label_weight[labels] + pos
  out    = concat([query_pos, query], -1).reshape(G*B, N, 512)

Sharding: data-parallel over the G*B=32 (g,b) pairs, 4 per core, 8 cores.
Each core computes its 4*2048=8192 points' `query` half on device; the
query_pos half is a passthrough assembled on the host.

Device layout (feature-major): per 512-point tile
  arg[128,3,512](PSUM)  = outer(s_k, prescaled_coords)   3 K=1 matmuls
  range-reduce arg to [-pi,pi] via int32 round-trip      DVE/GpSimd
  pe[128,3,512]         = Sin(arg + bias_vec)            1 ACT op (bias 0 / pi/2)
  h[128,4,512]          = relu(W1p @ pe + b1)            12 f32r matmuls + DVE
  q[128,2,512]          = W2 @ h + onehot.T@(lab_w+b2)   10 f32r matmuls accum
"""
import sys
import math

sys.path.insert(0, "/opt/trn_rl_repo")

import numpy as np
from contextlib import ExitStack

import concourse.bass as bass
import concourse.tile as tile
from concourse import bacc, library_config, mybir
from concourse.bass_utils import run_bass_kernel_spmd

# problem constants (hardcoded per contract)
G, B, N, F = 4, 8, 2048, 128
NCORES = 8
BPC = B * G // NCORES          # 4 (g,b) pairs per core
NPTS = BPC * N                 # 8192 points per core
T = 512                        # points per tile
NT = NPTS // T                 # 16 tiles
TWO_PI = 2.0 * math.pi
INV_TWO_PI = 1.0 / TWO_PI
F32 = mybir.dt.float32
F32R = mybir.dt.float32r
I32 = mybir.dt.int32

_CACHE = {}


def _build_program():
    nc = bacc.Bacc("TRN2", target_bir_lowering=False, debug=False, num_devices=NCORES)

    pc_d = nc.dram_tensor("pc", [NT, 1, 3, T], F32, kind="ExternalInput").ap()
    oh_d = nc.dram_tensor("oh", [NT, 10, T], F32R, kind="ExternalInput").ap()
    w1t_d = nc.dram_tensor("w1t", [3, 128, 512], F32R, kind="ExternalInput").ap()
    w2t_d = nc.dram_tensor("w2t", [4, 128, 256], F32R, kind="ExternalInput").ap()
    lwb_d = nc.dram_tensor("lwb", [10, 256], F32R, kind="ExternalInput").ap()
    svec_d = nc.dram_tensor("svec", [128, 1], F32, kind="ExternalInput").ap()
    sdiv_d = nc.dram_tensor("sdiv", [128, 1], F32, kind="ExternalInput").ap()
    invs2_d = nc.dram_tensor("invs2", [128, 1], F32, kind="ExternalInput").ap()
    bvec_d = nc.dram_tensor("bvec", [128, 1], F32, kind="ExternalInput").ap()
    b1c_d = nc.dram_tensor("b1c", [128, 4], F32, kind="ExternalInput").ap()
    q_d = nc.dram_tensor("q", [256, NPTS], F32, kind="ExternalOutput").ap()

    with tile.TileContext(nc) as tc, ExitStack() as ctx:
        cpool = ctx.enter_context(tc.tile_pool(name="consts", bufs=1))
        wpool = ctx.enter_context(tc.tile_pool(name="weights", bufs=1))
        io = ctx.enter_context(tc.tile_pool(name="io", bufs=3))
        work = ctx.enter_context(tc.tile_pool(name="work", bufs=2))
        psum_h = ctx.enter_context(tc.tile_pool(name="ph", bufs=1, space="PSUM"))
        psum_q = ctx.enter_context(tc.tile_pool(name="pq", bufs=2, space="PSUM"))

        nc.gpsimd.load_library(library_config.proxy)
        svec = cpool.tile([128, 1], F32)
        nc.sync.dma_start(svec[:], svec_d[:])
        sdiv = cpool.tile([128, 1], F32)
        nc.sync.dma_start(sdiv[:], sdiv_d[:])
        invs2 = cpool.tile([128, 1], F32)
        nc.sync.dma_start(invs2[:], invs2_d[:])
        bvec = cpool.tile([128, 1], F32)
        nc.sync.dma_start(bvec[:], bvec_d[:])
        b1c = cpool.tile([128, 4], F32)
        nc.sync.dma_start(b1c[:], b1c_d[:])
        lwb = cpool.tile([10, 256], F32R)
        nc.sync.dma_start(lwb[:], lwb_d[:])

        w1t = []
        for k in range(3):
            w = wpool.tile([128, 512], F32R, name=f"w1t{k}", tag=f"w1t{k}")
            nc.sync.dma_start(w[:], w1t_d[k])
            w1t.append(w)
        w2t = []
        for k in range(4):
            w = wpool.tile([128, 256], F32R, name=f"w2t{k}", tag=f"w2t{k}")
            nc.sync.dma_start(w[:], w2t_d[k])
            w2t.append(w)

        for t in range(NT):
            pc_t = io.tile([1, 3, T], F32, tag="pc_t")
            nc.sync.dma_start(pc_t[:], pc_d[t])
            oh_t = io.tile([10, T], F32R, tag="oh_t")
            nc.sync.dma_start(oh_t[:], oh_d[t])

            # ---- stage 1: broadcast prescaled coords across partitions (exact f32)
            bc = work.tile([128, 3, T], F32, tag="bc")
            for c in range(3):
                nc.gpsimd.partition_broadcast(bc[:, c, :], pc_t[:, c, :])

            # ---- stage 2: phase reduction in coordinate space:
            #   ki = round(bc * s/2pi);  bc2 = bc - ki * 2pi/s
            # then arg = s*bc2 = s*bc - 2pi*ki  lands in [-pi, pi]
            ki = work.tile([128, 3, T], I32, tag="ki")
            nc.vector.tensor_scalar(ki[:], bc[:], sdiv[:], None, op0=mybir.AluOpType.mult)
            kf = work.tile([128, 3, T], F32, tag="kf")
            nc.vector.tensor_scalar(kf[:], ki[:], invs2[:], None, op0=mybir.AluOpType.mult)
            bc2 = work.tile([128, 3, T], F32, tag="bc2")
            nc.gpsimd.tensor_sub(bc2[:], bc[:], kf[:])

            # ---- stage 3: pe = sin(s*bc2 + bias)  (rows 0:64 sin, 64:128 cos)
            pe = work.tile([128, 3, T], F32R, tag="pe")
            nc.scalar.activation(
                pe[:],
                bc2[:],
                mybir.ActivationFunctionType.Sin,
                bias=bvec[:],
                scale=svec[:],
            )

            # ---- stage 4: h = relu(W1p @ pe + b1), feature-major [4x128, T]
            hp = psum_h.tile([128, 4, T], F32, tag="hp")
            for m in range(4):
                for k in range(3):
                    nc.tensor.matmul(
                        hp[:, m, :],
                        w1t[k][:, m * 128 : (m + 1) * 128],
                        pe[:, k, :],
                        start=(k == 0),
                        stop=(k == 2),
                    )
            h = work.tile([128, 4, T], F32R, tag="h")
            for m in range(4):
                nc.scalar.activation(
                    h[:, m, :],
                    hp[:, m, :],
                    mybir.ActivationFunctionType.Relu,
                    bias=b1c[:, m : m + 1],
                )

            # ---- stage 5: q = W2 @ h + onehot^T-gather, feature-major [2x128, T]
            for mp in range(2):
                qp = psum_q.tile([128, T], F32, tag="qp")
                for k in range(4):
                    nc.tensor.matmul(
                        qp[:],
                        w2t[k][:, mp * 128 : (mp + 1) * 128],
                        h[:, k, :],
                        start=(k == 0),
                        stop=False,
                    )
                nc.tensor.matmul(
                    qp[:],
                    lwb[:, mp * 128 : (mp + 1) * 128],
                    oh_t[:],
                    start=False,
                    stop=True,
                )
                qs = work.tile([128, T], F32, tag="qs")
                nc.vector.tensor_copy(qs[:], qp[:])
                nc.sync.dma_start(q_d[mp * 128 : (mp + 1) * 128, t * T : (t + 1) * T], qs[:])

    nc.compile()
    return nc


def _host_prep(point_coord, labels, pc_range, noise, label_weight, W1, b1, W2, b2):
    """Build the per-core input maps (host-side sharding + weight prep)."""
    pc32 = np.asarray(point_coord, np.float32)
    lab = np.asarray(labels)
    noi = np.asarray(noise, np.float32)
    rng = np.asarray(pc_range, np.float32)

    small = (lab == 0) | (lab >= 6)
    std = np.where(small, 2.0, 4.0).astype(np.float32)            # [B,N]
    coords = pc32[None] + noi * std[None, :, :, None]             # [G,B,N,3]
    coords[0] = pc32                                              # group 0 originals
    low, high = rng[:3], rng[3:]
    pcs = (coords - low) / (high - low) * np.float32(TWO_PI)      # [G,B,N,3]
    pcs = pcs[..., [1, 0, 2]]   # reference concatenates pe in (y,x,z) order
    onehot = np.eye(10, dtype=np.float32)[np.asarray(lab, np.int64)]  # [B,N,10]

    # feature permutation: kernel row c*128+k -> ref feature c*128+2k (sin),
    # row c*128+64+k -> c*128+2k+1 (cos)
    perm = np.empty(3 * F, np.int64)
    for c in range(3):
        for k in range(64):
            perm[c * 128 + k] = c * 128 + 2 * k
            perm[c * 128 + 64 + k] = c * 128 + 2 * k + 1
    w1p = np.ascontiguousarray(np.asarray(W1, np.float32)[:, perm].T)  # [384,512]
    w2t = np.ascontiguousarray(np.asarray(W2, np.float32).T)           # [512,256]
    lwb = np.asarray(label_weight, np.float32) + np.asarray(b2, np.float32)[None]
    b1c = np.ascontiguousarray(np.asarray(b1, np.float32).reshape(4, 128).T)

    k64 = np.arange(64, dtype=np.float64)
    s64 = 10000.0 ** (-k64 / 64.0)
    s128 = np.concatenate([s64, s64])
    svec = s128.astype(np.float32).reshape(128, 1)
    sdiv = (s128 / (2 * np.pi)).astype(np.float32).reshape(128, 1)
    invs2 = (2 * np.pi / s128).astype(np.float32).reshape(128, 1)
    bvec = np.concatenate(
        [np.zeros(64, np.float32), np.full(64, np.pi / 2, np.float32)]
    ).reshape(128, 1)

    shared = {
        "w1t": w1p.reshape(3, 128, 512),
        "w2t": w2t.reshape(4, 128, 256),
        "lwb": np.ascontiguousarray(lwb),
        "svec": np.ascontiguousarray(svec),
        "sdiv": np.ascontiguousarray(sdiv),
        "invs2": np.ascontiguousarray(invs2),
        "bvec": np.ascontiguousarray(bvec),
        "b1c": b1c,
    }

    in_maps = []
    for core in range(NCORES):
        g = core // 2
        b0 = 4 * (core % 2)
        # [4b, N, 3] -> [3, NPTS] -> [3, NT, T] -> [NT, 3, T]
        pcc = pcs[g, b0 : b0 + 4].reshape(NPTS, 3).T
        pcc = np.ascontiguousarray(pcc.reshape(3, NT, T).transpose(1, 0, 2)).reshape(
            NT, 1, 3, T
        )
        ohc = onehot[b0 : b0 + 4].reshape(NPTS, 10).T
        ohc = np.ascontiguousarray(ohc.reshape(10, NT, T).transpose(1, 0, 2))
        in_maps.append({"pc": pcc, "oh": ohc, **shared})
    return in_maps


def _get_nc():
    if "nc" not in _CACHE:
        _CACHE["nc"] = _build_program()
    return _CACHE["nc"]


def _run_device(in_maps, trace=False, **kw):
    nc = _get_nc()
    return run_bass_kernel_spmd(nc, in_maps, list(range(NCORES)), trace=trace, **kw)


def kernel(point_coord, labels, pc_range, noise, query_pos, label_weight, W1, b1, W2, b2):
    in_maps = _host_prep(
        point_coord, labels, pc_range, noise, label_weight, W1, b1, W2, b2
    )
    res = _run_device(in_maps)

    qp = np.asarray(query_pos, np.float32)
    out = np.empty((G * B, N, 4 * F), np.float32)
    out[:, :, : 2 * F] = qp.reshape(G * B, N, 2 * F)
    for core in range(NCORES):
        q = res.results[core]["q"]                       # [256, NPTS]
        q = q.reshape(2 * F, BPC, N).transpose(1, 2, 0)  # [4, N, 256]
        out[4 * core : 4 * core + 4, :, 2 * F :] = q
    return out



# revision 18
# speedup vs baseline: 2.4498x; 2.4498x over previous
"""Trainium2 Bass kernel for nn_GroupPointEncoder.

Reference computation (G=4, B=8, N=2048, F=128):
  std = 2 or 4 per point by label class
  coords = [point_coord, (point_coord + noise*std)[1:]]           # [G,B,N,3]
  normed = (coords - low) / (high - low)
  pe     = interleaved sin/cos embedding, (y,x,z) order            # [G,B,N,384]
  h      = relu(pe @ W1.T + b1)                                    # [G,B,N,512]
  pos    = h @ W2.T + b2                                           # [G,B,N,256]
  query  = label_weight[labels] + pos
  out    = concat([query_pos, query], -1).reshape(G*B, N, 512)

Sharding: data-parallel over the G*B=32 (g,b) pairs, 4 per core, 8 cores.
Each core computes its 4*2048=8192 points' `query` half on device; the
query_pos half is a passthrough assembled on the host.

Device design (v2, fp8 DoubleRow):
  host ships pre-wrapped sine args v = wrap(s_k*c' + phi) as int8 (v*128/pi)
  ACT     pe = Sin(v * pi/128) -> fp8            [128,3,T] 1 inst
  Pool    pe chunk3 = copy of chunk0 (for DR pairing)
  PE      h = pe @ W1 via 12 fp8 DoubleRow matmuls (hi+lo residual weights)
  ACT/DVE Hs = max(h_psum + 32*b1, 0) -> fp8 (= 32*relu(...)), split 2+2
  PE      q_psum = Hs @ W2 (8 DR) + onehot-gather (2 DR), hi+lo residual
  DVE     q = q_psum * 1/1024 -> bf16, DMA out
All matmuls run as fp8e4 DoubleRow (0.5 cyc/row = 4x f32r throughput);
weight quantization error is cancelled by pairing each hi-chunk with its
residual lo-chunk inside the DoubleRow k-tile pairs.
"""
import sys
import math

sys.path.insert(0, "/opt/trn_rl_repo")

import numpy as np
import ml_dtypes
from contextlib import ExitStack

import concourse.bass as bass
import concourse.tile as tile
from concourse import bacc, library_config, mybir
from concourse.bass_utils import run_bass_kernel_spmd

# problem constants (hardcoded per contract)
G, B, N, F = 4, 8, 2048, 128
NCORES = 8
BPC = B * G // NCORES          # 4 (g,b) pairs per core
NPTS = BPC * N                 # 8192 points per core
T = 512                        # points per tile
NT = NPTS // T                 # 16 tiles
TWO_PI = 2.0 * math.pi
F32 = mybir.dt.float32
F16 = mybir.dt.float16
I8 = mybir.dt.int8
BF16 = mybir.dt.bfloat16
FP8 = mybir.dt.float8e4
E4 = ml_dtypes.float8_e4m3
DR = mybir.MatmulPerfMode.DoubleRow

_CACHE = {}


def _build_program():
    nc = bacc.Bacc("TRN2", target_bir_lowering=False, debug=False, num_devices=NCORES)

    args_d = nc.dram_tensor("args", [NT, 128, 3, T], I8, kind="ExternalInput").ap()
    oh_d = nc.dram_tensor("oh", [NT, 10, 2, T], FP8, kind="ExternalInput").ap()
    w1_d = nc.dram_tensor("w1", [128, 3, 2, 512], FP8, kind="ExternalInput").ap()
    w2_d = nc.dram_tensor("w2", [128, 4, 2, 256], FP8, kind="ExternalInput").ap()
    lwb_d = nc.dram_tensor("lwb", [10, 2, 2, 128], FP8, kind="ExternalInput").ap()
    b1c_d = nc.dram_tensor("b1c", [128, 4], F32, kind="ExternalInput").ap()
    q_d = nc.dram_tensor("q", [128, 2, NPTS], BF16, kind="ExternalOutput").ap()

    with tile.TileContext(nc) as tc, ExitStack() as ctx:
        cpool = ctx.enter_context(tc.tile_pool(name="consts", bufs=1))
        wpool = ctx.enter_context(tc.tile_pool(name="weights", bufs=1))
        io = ctx.enter_context(tc.tile_pool(name="io", bufs=3))
        work = ctx.enter_context(tc.tile_pool(name="work", bufs=2))
        psum_h = ctx.enter_context(tc.tile_pool(name="ph", bufs=1, space="PSUM"))
        psum_q = ctx.enter_context(tc.tile_pool(name="pq", bufs=2, space="PSUM"))

        b1c = cpool.tile([128, 4], F32)
        nc.sync.dma_start(b1c[:], b1c_d[:])
        lwb = cpool.tile([10, 2, 2, 128], FP8)
        nc.sync.dma_start(lwb[:], lwb_d[:])
        w1 = wpool.tile([128, 3, 2, 512], FP8)
        nc.sync.dma_start(w1[:], w1_d[:])
        w2 = wpool.tile([128, 4, 2, 256], FP8)
        nc.sync.dma_start(w2[:], w2_d[:])

        # software-pipelined: iteration t does stage-A work (sin, L1, relu)
        # for tile t and stage-B work (L2 + gather + evac) for tile t-1.
        prev = None  # (Hs, oh_t, t-1)
        for t in range(NT + 1):
            if t < NT:
                args_t = io.tile([128, 3, T], I8, tag="args")
                nc.sync.dma_start(args_t[:], args_d[t])
                oh_t = io.tile([10, 2, T], FP8, tag="oh")
                nc.sync.dma_start(oh_t[:], oh_d[t])

                # pe rows: sin(v*pi/128) exactly (v pre-wrapped on host);
                # chunk 3 duplicates chunk 0 so every DR pair is contiguous.
                pe = work.tile([128, 4, T], FP8, tag="pe")
                nc.scalar.activation(
                    pe[:, 0:3, :],
                    args_t[:],
                    mybir.ActivationFunctionType.Sin,
                    scale=float(np.pi / 128.0),
                )
                nc.gpsimd.tensor_copy(pe[:, 3, :], pe[:, 0, :])

                # L1: h = pe @ W1 (x32), K=384 in 3 DR pairs per out-block:
                #   d=0: (hi0,hi1) x pe(0,1); d=1: (hi2,lo0) x pe(2,0');
                #   d=2: (lo1,lo2) x pe(1,2)
                rhs1 = (pe[:, 0:2, :], pe[:, 2:4, :], pe[:, 1:3, :])
                h01 = psum_h.tile([128, 2, T], F32, tag="h01")
                h23 = psum_h.tile([128, 2, T], F32, tag="h23")
                Hs = work.tile([128, 4, T], FP8, tag="hs")
                for half, hp in ((0, h01), (1, h23)):
                    for m2 in range(2):
                        m = half * 2 + m2
                        for d in range(3):
                            nc.tensor.matmul(
                                hp[:, m2, :],
                                w1[:, d, :, m * 128 : (m + 1) * 128],
                                rhs1[d],
                                start=(d == 0),
                                stop=(d == 2),
                                perf_mode=DR,
                            )
                    # Hs = 32*relu(pe@W1 + b1): blocks 0,1 on ACT; 2,3 on DVE
                    for m2 in range(2):
                        m = half * 2 + m2
                        if half == 0:
                            nc.scalar.activation(
                                Hs[:, m, :],
                                hp[:, m2, :],
                                mybir.ActivationFunctionType.Relu,
                                bias=b1c[:, m : m + 1],
                            )
                        else:
                            nc.vector.tensor_scalar(
                                Hs[:, m, :],
                                hp[:, m2, :],
                                b1c[:, m : m + 1],
                                0.0,
                                op0=mybir.AluOpType.add,
                                op1=mybir.AluOpType.max,
                            )

            if prev is not None:
                Hp, ohp, tp = prev
                rhs2 = (Hp[:, 0:2, :], Hp[:, 2:4, :])
                qp = psum_q.tile([128, 2, T], F32, tag="qp")
                for mp in range(2):
                    for d in range(4):
                        nc.tensor.matmul(
                            qp[:, mp, :],
                            w2[:, d, :, mp * 128 : (mp + 1) * 128],
                            rhs2[d % 2],
                            start=(d == 0),
                            stop=False,
                            perf_mode=DR,
                        )
                    nc.tensor.matmul(
                        qp[:, mp, :],
                        lwb[:, mp, :, :],
                        ohp[:],
                        start=False,
                        stop=True,
                        perf_mode=DR,
                    )
                qs = work.tile([128, 2, T], BF16, tag="qs")
                nc.vector.tensor_scalar(
                    qs[:], qp[:], 1.0 / 1024.0, None, op0=mybir.AluOpType.mult
                )
                nc.sync.dma_start(q_d[:, :, tp * T : (tp + 1) * T], qs[:])

            if t < NT:
                prev = (Hs, oh_t, t)

    nc.compile()
    return nc


def _q8(x):
    return np.asarray(x, np.float32).astype(E4)


def _host_prep(point_coord, labels, pc_range, noise, label_weight, W1, b1, W2, b2):
    """Build the per-core input maps (host-side sharding + weight prep)."""
    pc32 = np.asarray(point_coord, np.float32)
    lab = np.asarray(labels)
    noi = np.asarray(noise, np.float32)
    rng = np.asarray(pc_range, np.float32)

    small = (lab == 0) | (lab >= 6)
    std = np.where(small, 2.0, 4.0).astype(np.float32)            # [B,N]
    coords = pc32[None] + noi * std[None, :, :, None]             # [G,B,N,3]
    coords[0] = pc32                                              # group 0 originals
    low, high = rng[:3], rng[3:]
    pcs = (coords - low) / (high - low) * np.float32(TWO_PI)      # [G,B,N,3]
    pcs = pcs[..., [1, 0, 2]]   # reference concatenates pe in (y,x,z) order
    onehot = np.eye(10, dtype=np.float32)[np.asarray(lab, np.int64)]  # [B,N,10]

    # feature permutation: kernel row c*128+k -> ref feature c*128+2k (sin),
    # row c*128+64+k -> c*128+2k+1 (cos)
    perm = np.empty(3 * F, np.int64)
    for c in range(3):
        for k in range(64):
            perm[c * 128 + k] = c * 128 + 2 * k
            perm[c * 128 + 64 + k] = c * 128 + 2 * k + 1

    k64 = np.arange(64, dtype=np.float64)
    s128 = np.concatenate([10000.0 ** (-k64 / 64.0)] * 2).astype(np.float64)
    phase = np.concatenate(
        [np.zeros(64, np.float64), np.full(64, np.pi / 2, np.float64)]
    )

    # W1 (x32), feature-permuted, transposed to [K=384, 512], hi+lo fp8 split
    A1 = (32.0 * np.asarray(W1, np.float32)[:, perm].T).astype(np.float32)
    a1 = [np.ascontiguousarray(A1[k * 128 : (k + 1) * 128]) for k in range(3)]
    hi1 = [_q8(a) for a in a1]
    lo1 = [_q8(a - h.astype(np.float32)) for a, h in zip(a1, hi1)]
    w1t = np.empty((128, 3, 2, 512), E4)
    w1t[:, 0, 0], w1t[:, 0, 1] = hi1[0], hi1[1]
    w1t[:, 1, 0], w1t[:, 1, 1] = hi1[2], lo1[0]
    w1t[:, 2, 0], w1t[:, 2, 1] = lo1[1], lo1[2]

    # W2 (x32), transposed to [K=512, 256], hi+lo fp8 split
    A2 = (32.0 * np.asarray(W2, np.float32).T).astype(np.float32)
    a2 = [np.ascontiguousarray(A2[k * 128 : (k + 1) * 128]) for k in range(4)]
    hi2 = [_q8(a) for a in a2]
    lo2 = [_q8(a - h.astype(np.float32)) for a, h in zip(a2, hi2)]
    w2t = np.empty((128, 4, 2, 256), E4)
    w2t[:, 0, 0], w2t[:, 0, 1] = hi2[0], hi2[1]
    w2t[:, 1, 0], w2t[:, 1, 1] = hi2[2], hi2[3]
    w2t[:, 2, 0], w2t[:, 2, 1] = lo2[0], lo2[1]
    w2t[:, 3, 0], w2t[:, 3, 1] = lo2[2], lo2[3]

    # label table (x64) + b2, hi+lo fp8; one-hot rhs carries the x16
    lwbs = 64.0 * (np.asarray(label_weight, np.float32) + np.asarray(b2, np.float32))
    lhi = _q8(lwbs)
    llo = _q8(lwbs - lhi.astype(np.float32))
    lwbt = np.empty((10, 2, 2, 128), E4)
    for mp in range(2):
        lwbt[:, mp, 0] = lhi[:, mp * 128 : (mp + 1) * 128]
        lwbt[:, mp, 1] = llo[:, mp * 128 : (mp + 1) * 128]

    b1c = np.ascontiguousarray(
        32.0 * np.asarray(b1, np.float32).reshape(4, 128).T
    )

    shared = {"w1": w1t, "w2": w2t, "lwb": lwbt, "b1c": b1c}

    in_maps = []
    for core in range(NCORES):
        g = core // 2
        b0 = 4 * (core % 2)
        # pre-wrapped sine arguments: v[p, c, n] = wrap(s_p * c' + phi_p),
        # quantized to int8 turns-of-pi/128 (device computes sin(v*pi/128))
        pcc = pcs[g, b0 : b0 + 4].reshape(NPTS, 3).T.astype(np.float64)  # [3,NPTS]
        ang = s128[:, None, None] * pcc[None] + phase[:, None, None]
        ang = np.mod(ang + np.pi, TWO_PI) - np.pi
        ang = np.clip(np.rint(ang * (128.0 / np.pi)), -128, 127).astype(np.int8)
        ang = np.ascontiguousarray(
            ang.reshape(128, 3, NT, T).transpose(2, 0, 1, 3)
        )                                                           # [NT,128,3,T]

        ohc = onehot[b0 : b0 + 4].reshape(NPTS, 10).T               # [10,NPTS]
        ohc16 = (16.0 * ohc).astype(E4).reshape(10, NT, T)
        ohp = np.empty((NT, 10, 2, T), E4)
        ohp[:, :, 0] = ohc16.transpose(1, 0, 2)
        ohp[:, :, 1] = ohp[:, :, 0]
        in_maps.append({"args": ang, "oh": ohp, **shared})
    return in_maps


def _get_nc():
    if "nc" not in _CACHE:
        _CACHE["nc"] = _build_program()
    return _CACHE["nc"]


def _run_device(in_maps, trace=False, **kw):
    nc = _get_nc()
    return run_bass_kernel_spmd(nc, in_maps, list(range(NCORES)), trace=trace, **kw)


def kernel(point_coord, labels, pc_range, noise, query_pos, label_weight, W1, b1, W2, b2):
    in_maps = _host_prep(
        point_coord, labels, pc_range, noise, label_weight, W1, b1, W2, b2
    )
    res = _run_device(in_maps)

    qp = np.asarray(query_pos, np.float32)
    out = np.empty((G * B, N, 4 * F), np.float32)
    out[:, :, : 2 * F] = qp.reshape(G * B, N, 2 * F)
    for core in range(NCORES):
        q = np.asarray(res.results[core]["q"], np.float32)  # [128,2,NPTS]
        q = q.transpose(1, 0, 2).reshape(2 * F, BPC, N).transpose(1, 2, 0)
        out[4 * core : 4 * core + 4, :, 2 * F :] = q        # [4, N, 256]
    return out


# revision 19
# speedup vs baseline: 3.1747x; 1.2959x over previous
"""Trainium2 Bass kernel for nn_GroupPointEncoder.

Reference computation (G=4, B=8, N=2048, F=128):
  std = 2 or 4 per point by label class
  coords = [point_coord, (point_coord + noise*std)[1:]]           # [G,B,N,3]
  normed = (coords - low) / (high - low)
  pe     = interleaved sin/cos embedding, (y,x,z) order            # [G,B,N,384]
  h      = relu(pe @ W1.T + b1)                                    # [G,B,N,512]
  pos    = h @ W2.T + b2                                           # [G,B,N,256]
  query  = label_weight[labels] + pos
  out    = concat([query_pos, query], -1).reshape(G*B, N, 512)

Sharding: data-parallel over the G*B=32 (g,b) pairs, 4 per core, 8 cores.
Each core computes its 4*2048=8192 points' `query` half on device; the
query_pos half is a passthrough assembled on the host.

Device design (v3, quadratic tail collapse):
  For frequencies k >= 32, s_k <= 1e-2 so |s_k*c'| <= 0.12 rad and
  sin(t) = t, cos(t) = 1 - t^2/2 to ~2.5e-4. Those 192 features fold into
  7 exact K-rows: [y, y^2, x, x^2, z, z^2, 1] with host-folded W1
  coefficients (the 1-row also absorbs b1). K shrinks 384 -> 199, packed
  into 2 chunks of 128 (rows 71..127 of chunk B have zero W1 rows).

  host ships: sine args for kept features (int8 * pi/128), quad rows
  (bf16), one-hot (bf16), W1 (bf16, x32), W2 (fp8 hi, x32),
  label table (bf16, x1024, +b2)
  ACT   pe chunk A/B = Sin(args * pi/128) -> bf16 (2 insts)
  DMA   pe[64:71, chunk B] <- quad rows (after sin)
  PE    h = pe @ W1: 2 bf16 matmuls per out-block (8 total)
  ACT/DVE Hs = max(h_psum, 0) -> fp8 (b1 folded into the 1-row), 1+1 insts
  PE    q = Hs @ W2 (4 fp8 DoubleRow) + onehot @ lwb (2 bf16), x1024
  DVE   qs = q_psum * 1/1024 -> bf16; DMA out
"""
import sys
import math

sys.path.insert(0, "/opt/trn_rl_repo")

import numpy as np
import ml_dtypes
from contextlib import ExitStack

import concourse.bass as bass
import concourse.tile as tile
from concourse import bacc, library_config, mybir
from concourse.bass_utils import run_bass_kernel_spmd

# problem constants (hardcoded per contract)
G, B, N, F = 4, 8, 2048, 128
NCORES = 8
BPC = B * G // NCORES          # 4 (g,b) pairs per core
NPTS = BPC * N                 # 8192 points per core
T = 512                        # points per tile
NT = NPTS // T                 # 16 tiles
KK = 32                        # kept frequencies per coord (k < KK exact sin)
TWO_PI = 2.0 * math.pi
F32 = mybir.dt.float32
I8 = mybir.dt.int8
BF16 = mybir.dt.bfloat16
FP8 = mybir.dt.float8e4
E4 = ml_dtypes.float8_e4m3
BF = ml_dtypes.bfloat16
DR = mybir.MatmulPerfMode.DoubleRow

_CACHE = {}


def _build_program():
    nc = bacc.Bacc("TRN2", target_bir_lowering=False, debug=False, num_devices=NCORES)

    args_d = nc.dram_tensor("args", [NT, 2, 128, T], I8, kind="ExternalInput").ap()
    quad_d = nc.dram_tensor("quad", [NT, 7, T], BF16, kind="ExternalInput").ap()
    oh_d = nc.dram_tensor("oh", [NT, 10, T], BF16, kind="ExternalInput").ap()
    w1_d = nc.dram_tensor("w1", [128, 2, 512], BF16, kind="ExternalInput").ap()
    w2_d = nc.dram_tensor("w2", [128, 2, 2, 256], FP8, kind="ExternalInput").ap()
    lwb_d = nc.dram_tensor("lwb", [10, 256], BF16, kind="ExternalInput").ap()
    q_d = nc.dram_tensor("q", [128, 2, NPTS], BF16, kind="ExternalOutput").ap()

    with tile.TileContext(nc) as tc, ExitStack() as ctx:
        wpool = ctx.enter_context(tc.tile_pool(name="weights", bufs=1))
        io = ctx.enter_context(tc.tile_pool(name="io", bufs=3))
        work = ctx.enter_context(tc.tile_pool(name="work", bufs=2))
        psum_h = ctx.enter_context(tc.tile_pool(name="ph", bufs=1, space="PSUM"))
        psum_q = ctx.enter_context(tc.tile_pool(name="pq", bufs=2, space="PSUM"))

        lwb = wpool.tile([10, 256], BF16)
        nc.sync.dma_start(lwb[:], lwb_d[:])
        w1 = wpool.tile([128, 2, 512], BF16)
        nc.sync.dma_start(w1[:], w1_d[:])
        w2 = wpool.tile([128, 2, 2, 256], FP8)
        nc.sync.dma_start(w2[:], w2_d[:])

        # software-pipelined: iteration t does stage-A work (sin, L1, relu)
        # for tile t and stage-B work (L2 + gather + evac) for tile t-1.
        prev = None  # (Hs, oh_t, t-1)
        for t in range(NT + 1):
            if t < NT:
                args_a = io.tile([128, T], I8, tag="argsa")
                nc.sync.dma_start(args_a[:], args_d[t, 0])
                args_b = io.tile([128, T], I8, tag="argsb")
                nc.sync.dma_start(args_b[:], args_d[t, 1])
                oh_t = io.tile([10, T], BF16, tag="oh")
                nc.sync.dma_start(oh_t[:], oh_d[t])

                # pe chunk A = kept y/x features; chunk B = kept z features
                # (rows 0..63), quad rows (64..70, DMA'd after sin), rows
                # 71..127 are sin(garbage) but their W1 rows are zero.
                pe = work.tile([128, 2, T], BF16, tag="pe")
                nc.scalar.activation(
                    pe[:, 0, :],
                    args_a[:],
                    mybir.ActivationFunctionType.Sin,
                    scale=float(np.pi / 128.0),
                )
                nc.scalar.activation(
                    pe[:, 1, :],
                    args_b[:],
                    mybir.ActivationFunctionType.Sin,
                    scale=float(np.pi / 128.0),
                )
                nc.sync.dma_start(pe[64:71, 1, :], quad_d[t])

                # L1: h = pe @ W1 (x32, b1 folded into the ones-row)
                h01 = psum_h.tile([128, 2, T], F32, tag="h01")
                h23 = psum_h.tile([128, 2, T], F32, tag="h23")
                Hs = work.tile([128, 4, T], FP8, tag="hs")
                for half, hp in ((0, h01), (1, h23)):
                    for m2 in range(2):
                        m = half * 2 + m2
                        for kc in range(2):
                            nc.tensor.matmul(
                                hp[:, m2, :],
                                w1[:, kc, m * 128 : (m + 1) * 128],
                                pe[:, kc, :],
                                start=(kc == 0),
                                stop=(kc == 1),
                            )
                # Hs = 32*relu(...) -> fp8; blocks 0,1 on ACT; 2,3 on DVE
                nc.scalar.activation(
                    Hs[:, 0:2, :], h01[:], mybir.ActivationFunctionType.Relu
                )
                nc.vector.tensor_scalar(
                    Hs[:, 2:4, :], h23[:], 0.0, None, op0=mybir.AluOpType.max
                )

            if prev is not None:
                Hp, ohp, tp = prev
                rhs2 = (Hp[:, 0:2, :], Hp[:, 2:4, :])
                qp = psum_q.tile([128, 2, T], F32, tag="qp")
                for mp in range(2):
                    for d in range(2):
                        nc.tensor.matmul(
                            qp[:, mp, :],
                            w2[:, d, :, mp * 128 : (mp + 1) * 128],
                            rhs2[d],
                            start=(d == 0),
                            stop=False,
                            perf_mode=DR,
                        )
                    nc.tensor.matmul(
                        qp[:, mp, :],
                        lwb[:, mp * 128 : (mp + 1) * 128],
                        ohp[:],
                        start=False,
                        stop=True,
                    )
                qs = work.tile([128, 2, T], BF16, tag="qs")
                nc.vector.tensor_scalar(
                    qs[:], qp[:], 1.0 / 1024.0, None, op0=mybir.AluOpType.mult
                )
                nc.sync.dma_start(q_d[:, :, tp * T : (tp + 1) * T], qs[:])

            if t < NT:
                prev = (Hs, oh_t, t)

    nc.compile()
    return nc


def _host_prep(point_coord, labels, pc_range, noise, label_weight, W1, b1, W2, b2):
    """Build the per-core input maps (host-side sharding + weight prep)."""
    pc32 = np.asarray(point_coord, np.float32)
    lab = np.asarray(labels)
    noi = np.asarray(noise, np.float32)
    rng = np.asarray(pc_range, np.float32)

    small = (lab == 0) | (lab >= 6)
    std = np.where(small, 2.0, 4.0).astype(np.float32)            # [B,N]
    coords = pc32[None] + noi * std[None, :, :, None]             # [G,B,N,3]
    coords[0] = pc32                                              # group 0 originals
    low, high = rng[:3], rng[3:]
    pcs = (coords - low) / (high - low) * np.float32(TWO_PI)      # [G,B,N,3]
    pcs = pcs[..., [1, 0, 2]]   # reference concatenates pe in (y,x,z) order
    onehot = np.eye(10, dtype=np.float32)[np.asarray(lab, np.int64)]  # [B,N,10]

    W1f = np.asarray(W1, np.float32)    # [512, 384]
    b1f = np.asarray(b1, np.float32)
    kk = np.arange(64, dtype=np.float64)
    s64 = 10000.0 ** (-kk / 64.0)       # s_k

    # --- W1 chunk A: kept y/x features; chunk B: kept z + quad rows ---
    # partition p of chunk A: p<32 y-sin k=p; 32..63 y-cos; 64..95 x-sin;
    # 96..127 x-cos. chunk B: 0..31 z-sin; 32..63 z-cos; 64..70 quad rows
    # [y, y^2, x, x^2, z, z^2, 1]; 71..127 zero.
    def feat_idx(c, k, cos):
        return c * 128 + 2 * k + (1 if cos else 0)

    featA = np.empty(128, np.int64)
    featA[0:32] = [feat_idx(0, k, False) for k in range(KK)]
    featA[32:64] = [feat_idx(0, k, True) for k in range(KK)]
    featA[64:96] = [feat_idx(1, k, False) for k in range(KK)]
    featA[96:128] = [feat_idx(1, k, True) for k in range(KK)]
    featBz = np.empty(64, np.int64)
    featBz[0:32] = [feat_idx(2, k, False) for k in range(KK)]
    featBz[32:64] = [feat_idx(2, k, True) for k in range(KK)]

    w1t = np.zeros((128, 2, 512), np.float32)
    w1t[:, 0, :] = 32.0 * W1f[:, featA].T
    w1t[0:64, 1, :] = 32.0 * W1f[:, featBz].T
    s_tail = s64[KK:]                                   # [32..63]
    const_acc = b1f.copy()
    for c in range(3):
        sin_cols = W1f[:, [feat_idx(c, k, False) for k in range(KK, 64)]]  # [512,32]
        cos_cols = W1f[:, [feat_idx(c, k, True) for k in range(KK, 64)]]
        w1t[64 + 2 * c, 1, :] = 32.0 * (sin_cols @ s_tail)            # * c'
        w1t[65 + 2 * c, 1, :] = 32.0 * (cos_cols @ (-0.5 * s_tail**2))  # * c'^2
        const_acc = const_acc + cos_cols.sum(axis=1)
    w1t[70, 1, :] = 32.0 * const_acc                                  # * 1
    w1t = w1t.astype(BF)

    # --- W2 (x32) -> fp8 hi, DoubleRow pairs (k0,k1),(k2,k3) ---
    A2 = (32.0 * np.asarray(W2, np.float32).T).astype(np.float32)     # [512,256]
    w2t = np.empty((128, 2, 2, 256), E4)
    for d in range(2):
        for i in range(2):
            k = 2 * d + i
            w2t[:, d, i] = A2[k * 128 : (k + 1) * 128].astype(E4)

    # --- label table (x1024, +b2), bf16 ---
    lwbt = (
        1024.0 * (np.asarray(label_weight, np.float32) + np.asarray(b2, np.float32))
    ).astype(BF)

    shared = {"w1": w1t, "w2": w2t, "lwb": np.ascontiguousarray(lwbt)}

    phase_pat = np.concatenate(
        [np.zeros(KK), np.full(KK, np.pi / 2)]
    )                                                   # per 64-row block
    s_pat = np.concatenate([s64[:KK], s64[:KK]])        # [64]

    in_maps = []
    for core in range(NCORES):
        g = core // 2
        b0 = 4 * (core % 2)
        pcc = pcs[g, b0 : b0 + 4].reshape(NPTS, 3).T.astype(np.float64)  # [3,NPTS]

        # sine args for kept features, int8 units of pi/128
        ang = np.zeros((2, 128, NPTS), np.float64)
        ang[0, 0:64] = s_pat[:, None] * pcc[0][None] + phase_pat[:, None]   # y
        ang[0, 64:128] = s_pat[:, None] * pcc[1][None] + phase_pat[:, None]  # x
        ang[1, 0:64] = s_pat[:, None] * pcc[2][None] + phase_pat[:, None]   # z
        ang = np.mod(ang + np.pi, TWO_PI) - np.pi
        ang = np.clip(np.rint(ang * (128.0 / np.pi)), -128, 127).astype(np.int8)
        ang = np.ascontiguousarray(
            ang.reshape(2, 128, NT, T).transpose(2, 0, 1, 3)
        )                                               # [NT,2,128,T]

        quad = np.empty((7, NPTS), np.float32)
        quad[0], quad[2], quad[4] = pcc[0], pcc[1], pcc[2]
        quad[1], quad[3], quad[5] = pcc[0] ** 2, pcc[1] ** 2, pcc[2] ** 2
        quad[6] = 1.0
        quad = np.ascontiguousarray(
            quad.astype(BF).reshape(7, NT, T).transpose(1, 0, 2)
        )                                               # [NT,7,T]

        ohc = onehot[b0 : b0 + 4].reshape(NPTS, 10).T.astype(BF)  # [10,NPTS]
        ohc = np.ascontiguousarray(ohc.reshape(10, NT, T).transpose(1, 0, 2))
        in_maps.append({"args": ang, "quad": quad, "oh": ohc, **shared})
    return in_maps


def _get_nc():
    if "nc" not in _CACHE:
        _CACHE["nc"] = _build_program()
    return _CACHE["nc"]


def _run_device(in_maps, trace=False, **kw):
    nc = _get_nc()
    return run_bass_kernel_spmd(nc, in_maps, list(range(NCORES)), trace=trace, **kw)


def kernel(point_coord, labels, pc_range, noise, query_pos, label_weight, W1, b1, W2, b2):
    in_maps = _host_prep(
        point_coord, labels, pc_range, noise, label_weight, W1, b1, W2, b2
    )
    res = _run_device(in_maps)

    qp = np.asarray(query_pos, np.float32)
    out = np.empty((G * B, N, 4 * F), np.float32)
    out[:, :, : 2 * F] = qp.reshape(G * B, N, 2 * F)
    for core in range(NCORES):
        q = np.asarray(res.results[core]["q"], np.float32)  # [128,2,NPTS]
        q = q.transpose(1, 0, 2).reshape(2 * F, BPC, N).transpose(1, 2, 0)
        out[4 * core : 4 * core + 4, :, 2 * F :] = q        # [4, N, 256]
    return out


# revision 20
# speedup vs baseline: 3.6754x; 1.1577x over previous
"""Trainium2 Bass kernel for nn_GroupPointEncoder.

Reference computation (G=4, B=8, N=2048, F=128):
  std = 2 or 4 per point by label class
  coords = [point_coord, (point_coord + noise*std)[1:]]           # [G,B,N,3]
  normed = (coords - low) / (high - low)
  pe     = interleaved sin/cos embedding, (y,x,z) order            # [G,B,N,384]
  h      = relu(pe @ W1.T + b1)                                    # [G,B,N,512]
  pos    = h @ W2.T + b2                                           # [G,B,N,256]
  query  = label_weight[labels] + pos
  out    = concat([query_pos, query], -1).reshape(G*B, N, 512)

Sharding: data-parallel over the G*B=32 (g,b) pairs, 4 per core, 8 cores.
Each core computes its 4*2048=8192 points' `query` half on device; the
query_pos half is a passthrough assembled on the host.

Device design (v4, single-chunk K via 8th-order tail collapse):
  High frequencies have |s_k * c'| small, so sin/cos Taylor-expand in c'.
  Keeping k < KK exact per coord (KK=11 for x/y, 27 for z whose pc_range
  is tiny vs the data spread) and folding the tails into 25 polynomial
  K-rows [c, c^2, .., c^8 per coord, 1] (odd powers serve sin, even cos;
  the 1-row absorbs all cos constants AND b1), K drops 384 -> 123 <= 128:
  L1 is ONE bf16 matmul per out-block.

  host ships: sine args (int8 * pi/128) for the 98 kept rows, poly rows
  (bf16), one-hot (bf16), W1 (bf16, x32, tail-folded), W2 (fp8 hi, x32),
  label table (bf16, x1024, +b2)
  ACT   pe[0:114] = Sin(args * pi/128) -> bf16 (1 inst)
  DMA   pe[114:128] <- poly rows (independent of sin)
  PE    h = pe @ W1: 1 bf16 matmul per out-block (4 total)
  ACT/DVE Hs = max(h_psum, 0) -> fp8 (= 32*relu), 1+1 insts
  PE    q = Hs @ W2 (4 fp8 DoubleRow) + onehot @ lwb (2 bf16), x1024
  DVE   qs = q_psum * 1/1024 -> bf16; DMA out
"""
import sys
import math

sys.path.insert(0, "/opt/trn_rl_repo")

import numpy as np
import ml_dtypes
from contextlib import ExitStack

import concourse.bass as bass
import concourse.tile as tile
from concourse import bacc, library_config, mybir
from concourse.bass_utils import run_bass_kernel_spmd

# problem constants (hardcoded per contract)
G, B, N, F = 4, 8, 2048, 128
NCORES = 8
BPC = B * G // NCORES          # 4 (g,b) pairs per core
NPTS = BPC * N                 # 8192 points per core
T = 512                        # points per tile
NT = NPTS // T                 # 16 tiles
KKXY, KKZ = 11, 27             # kept frequencies (exact sin) per coord
NKEPT = 4 * KKXY + 2 * KKZ     # 98 kept feature rows
NPOLY = 25                     # 8 powers x 3 coords + ones row
TWO_PI = 2.0 * math.pi
F32 = mybir.dt.float32
I8 = mybir.dt.int8
BF16 = mybir.dt.bfloat16
FP8 = mybir.dt.float8e4
E4 = ml_dtypes.float8_e4m3
BF = ml_dtypes.bfloat16
DR = mybir.MatmulPerfMode.DoubleRow

_CACHE = {}


def _build_program():
    nc = bacc.Bacc("TRN2", target_bir_lowering=False, debug=False, num_devices=NCORES)

    args_d = nc.dram_tensor("args", [NT, NKEPT, T], I8, kind="ExternalInput").ap()
    poly_d = nc.dram_tensor("poly", [NT, 128 - NKEPT, T], BF16, kind="ExternalInput").ap()
    oh_d = nc.dram_tensor("oh", [NT, 10, T], BF16, kind="ExternalInput").ap()
    w1_d = nc.dram_tensor("w1", [128, 512], BF16, kind="ExternalInput").ap()
    w2_d = nc.dram_tensor("w2", [128, 2, 2, 256], FP8, kind="ExternalInput").ap()
    lwb_d = nc.dram_tensor("lwb", [10, 256], BF16, kind="ExternalInput").ap()
    q_d = nc.dram_tensor("q", [128, 2, NPTS], BF16, kind="ExternalOutput").ap()

    with tile.TileContext(nc) as tc, ExitStack() as ctx:
        wpool = ctx.enter_context(tc.tile_pool(name="weights", bufs=1))
        io = ctx.enter_context(tc.tile_pool(name="io", bufs=3))
        work = ctx.enter_context(tc.tile_pool(name="work", bufs=2))
        psum_h = ctx.enter_context(tc.tile_pool(name="ph", bufs=1, space="PSUM"))
        psum_q = ctx.enter_context(tc.tile_pool(name="pq", bufs=2, space="PSUM"))

        lwb = wpool.tile([10, 256], BF16)
        nc.sync.dma_start(lwb[:], lwb_d[:])
        w1 = wpool.tile([128, 512], BF16)
        nc.sync.dma_start(w1[:], w1_d[:])
        w2 = wpool.tile([128, 2, 2, 256], FP8)
        nc.sync.dma_start(w2[:], w2_d[:])

        # software-pipelined: iteration t does stage-A work (sin, L1, relu)
        # for tile t and stage-B work (L2 + gather + evac) for tile t-1.
        prev = None  # (Hs, oh_t, t-1)
        for t in range(NT + 1):
            if t < NT:
                args_t = io.tile([NKEPT, T], I8, tag="args")
                nc.sync.dma_start(args_t[:], args_d[t])
                oh_t = io.tile([10, T], BF16, tag="oh")
                nc.sync.dma_start(oh_t[:], oh_d[t])

                # pe rows 0..97: exact sin features; 98..127: poly rows via
                # DMA (no dependency on the sin instruction).
                pe = work.tile([128, T], BF16, tag="pe")
                nc.scalar.activation(
                    pe[0:NKEPT, :],
                    args_t[:],
                    mybir.ActivationFunctionType.Sin,
                    scale=float(np.pi / 128.0),
                )
                nc.sync.dma_start(pe[NKEPT:128, :], poly_d[t])

                # L1: h = pe @ W1 (x32; b1 + cos constants folded into the
                # ones row), one K=128 bf16 matmul per out-block
                h01 = psum_h.tile([128, 2, T], F32, tag="h01")
                h23 = psum_h.tile([128, 2, T], F32, tag="h23")
                Hs = work.tile([128, 4, T], FP8, tag="hs")
                for half, hp in ((0, h01), (1, h23)):
                    for m2 in range(2):
                        m = half * 2 + m2
                        nc.tensor.matmul(
                            hp[:, m2, :],
                            w1[:, m * 128 : (m + 1) * 128],
                            pe[:],
                            start=True,
                            stop=True,
                        )
                # Hs = 32*relu(...) -> fp8; blocks 0,1 on ACT; 2,3 on DVE
                nc.scalar.activation(
                    Hs[:, 0:2, :], h01[:], mybir.ActivationFunctionType.Relu
                )
                nc.vector.tensor_scalar(
                    Hs[:, 2:4, :], h23[:], 0.0, None, op0=mybir.AluOpType.max
                )

            if prev is not None:
                Hp, ohp, tp = prev
                rhs2 = (Hp[:, 0:2, :], Hp[:, 2:4, :])
                qp = psum_q.tile([128, 2, T], F32, tag="qp")
                for mp in range(2):
                    for d in range(2):
                        nc.tensor.matmul(
                            qp[:, mp, :],
                            w2[:, d, :, mp * 128 : (mp + 1) * 128],
                            rhs2[d],
                            start=(d == 0),
                            stop=False,
                            perf_mode=DR,
                        )
                    nc.tensor.matmul(
                        qp[:, mp, :],
                        lwb[:, mp * 128 : (mp + 1) * 128],
                        ohp[:],
                        start=False,
                        stop=True,
                    )
                qs = work.tile([128, 2, T], BF16, tag="qs")
                nc.vector.tensor_scalar(
                    qs[:], qp[:], 1.0 / 1024.0, None, op0=mybir.AluOpType.mult
                )
                nc.sync.dma_start(q_d[:, :, tp * T : (tp + 1) * T], qs[:])

            if t < NT:
                prev = (Hs, oh_t, t)

    nc.compile()
    return nc


def _row_plan():
    """Row layout of the single K chunk: per-coord kept (sin,cos) runs,
    then poly rows [c^1..c^8] per coord, then the ones row."""
    kks = (KKXY, KKXY, KKZ)
    starts = []
    off = 0
    for c in range(3):
        starts.append(off)
        off += 2 * kks[c]
    poly_base = off           # == NKEPT
    return kks, starts, poly_base


def _host_prep(point_coord, labels, pc_range, noise, label_weight, W1, b1, W2, b2):
    """Build the per-core input maps (host-side sharding + weight prep)."""
    pc32 = np.asarray(point_coord, np.float32)
    lab = np.asarray(labels)
    noi = np.asarray(noise, np.float32)
    rng = np.asarray(pc_range, np.float32)

    small = (lab == 0) | (lab >= 6)
    std = np.where(small, 2.0, 4.0).astype(np.float32)            # [B,N]
    coords = pc32[None] + noi * std[None, :, :, None]             # [G,B,N,3]
    coords[0] = pc32                                              # group 0 originals
    low, high = rng[:3], rng[3:]
    pcs = (coords - low) / (high - low) * np.float32(TWO_PI)      # [G,B,N,3]
    pcs = pcs[..., [1, 0, 2]]   # reference concatenates pe in (y,x,z) order
    onehot = np.eye(10, dtype=np.float32)[np.asarray(lab, np.int64)]  # [B,N,10]

    W1f = np.asarray(W1, np.float32)    # [512, 384]
    b1f = np.asarray(b1, np.float32)
    kk64 = np.arange(64, dtype=np.float64)
    s64 = 10000.0 ** (-kk64 / 64.0)

    def fi(c, k, cos):
        return c * 128 + 2 * k + (1 if cos else 0)

    kks, starts, poly_base = _row_plan()

    # --- W1 single chunk (x32): kept rows + folded polynomial tail ---
    w1t = np.zeros((128, 512), np.float32)
    const_acc = b1f.astype(np.float64).copy()
    for c in range(3):
        kk = kks[c]
        st = starts[c]
        sin_idx = [fi(c, k, False) for k in range(kk)]
        cos_idx = [fi(c, k, True) for k in range(kk)]
        w1t[st : st + kk] = 32.0 * W1f[:, sin_idx].T
        w1t[st + kk : st + 2 * kk] = 32.0 * W1f[:, cos_idx].T
        s_t = s64[kk:]
        sc = W1f[:, [fi(c, k, False) for k in range(kk, 64)]].astype(np.float64)
        cc = W1f[:, [fi(c, k, True) for k in range(kk, 64)]].astype(np.float64)
        for p in range(1, 9):
            fac = math.factorial(p)
            if p % 2 == 1:
                sign = -1.0 if (p - 1) // 2 % 2 else 1.0
                coef = sc @ (sign * s_t**p / fac)
            else:
                sign = -1.0 if (p // 2) % 2 else 1.0
                coef = cc @ (sign * s_t**p / fac)
            w1t[poly_base + 8 * c + (p - 1)] = 32.0 * coef
        const_acc += cc.sum(axis=1)
    w1t[poly_base + 24] = 32.0 * const_acc
    w1t = w1t.astype(BF)

    # --- W2 (x32) -> fp8 hi, DoubleRow pairs (k0,k1),(k2,k3) ---
    A2 = (32.0 * np.asarray(W2, np.float32).T).astype(np.float32)     # [512,256]
    w2t = np.empty((128, 2, 2, 256), E4)
    for d in range(2):
        for i in range(2):
            k = 2 * d + i
            w2t[:, d, i] = A2[k * 128 : (k + 1) * 128].astype(E4)

    # --- label table (x1024, +b2), bf16 ---
    lwbt = (
        1024.0 * (np.asarray(label_weight, np.float32) + np.asarray(b2, np.float32))
    ).astype(BF)

    shared = {"w1": w1t, "w2": w2t, "lwb": np.ascontiguousarray(lwbt)}

    in_maps = []
    for core in range(NCORES):
        g = core // 2
        b0 = 4 * (core % 2)
        pcc = pcs[g, b0 : b0 + 4].reshape(NPTS, 3).T.astype(np.float64)  # [3,NPTS]

        # sine args for kept rows, int8 units of pi/128
        ang = np.empty((NKEPT, NPTS), np.float64)
        for c in range(3):
            kk = kks[c]
            st = starts[c]
            sv = s64[:kk]
            ang[st : st + kk] = sv[:, None] * pcc[c][None]
            ang[st + kk : st + 2 * kk] = sv[:, None] * pcc[c][None] + np.pi / 2
        ang = np.mod(ang + np.pi, TWO_PI) - np.pi
        ang = np.clip(np.rint(ang * (128.0 / np.pi)), -128, 127).astype(np.int8)
        ang = np.ascontiguousarray(
            ang.reshape(NKEPT, NT, T).transpose(1, 0, 2)
        )                                               # [NT,NKEPT,T]

        poly = np.zeros((128 - NKEPT, NPTS), np.float32)
        for c in range(3):
            v = np.ones_like(pcc[c])
            for p in range(8):
                v = v * pcc[c]
                poly[8 * c + p] = v
        poly[24] = 1.0
        poly = np.ascontiguousarray(
            poly.astype(BF).reshape(128 - NKEPT, NT, T).transpose(1, 0, 2)
        )                                               # [NT,30,T]

        ohc = onehot[b0 : b0 + 4].reshape(NPTS, 10).T.astype(BF)  # [10,NPTS]
        ohc = np.ascontiguousarray(ohc.reshape(10, NT, T).transpose(1, 0, 2))
        in_maps.append({"args": ang, "poly": poly, "oh": ohc, **shared})
    return in_maps


def _get_nc():
    if "nc" not in _CACHE:
        _CACHE["nc"] = _build_program()
    return _CACHE["nc"]


def _run_device(in_maps, trace=False, **kw):
    nc = _get_nc()
    return run_bass_kernel_spmd(nc, in_maps, list(range(NCORES)), trace=trace, **kw)


def kernel(point_coord, labels, pc_range, noise, query_pos, label_weight, W1, b1, W2, b2):
    in_maps = _host_prep(
        point_coord, labels, pc_range, noise, label_weight, W1, b1, W2, b2
    )
    res = _run_device(in_maps)

    qp = np.asarray(query_pos, np.float32)
    out = np.empty((G * B, N, 4 * F), np.float32)
    out[:, :, : 2 * F] = qp.reshape(G * B, N, 2 * F)
    for core in range(NCORES):
        q = np.asarray(res.results[core]["q"], np.float32)  # [128,2,NPTS]
        q = q.transpose(1, 0, 2).reshape(2 * F, BPC, N).transpose(1, 2, 0)
        out[4 * core : 4 * core + 4, :, 2 * F :] = q        # [4, N, 256]
    return out


# revision 21
# speedup vs baseline: 3.7285x; 1.0144x over previous
"""Trainium2 Bass kernel for nn_GroupPointEncoder.

Reference computation (G=4, B=8, N=2048, F=128):
  std = 2 or 4 per point by label class
  coords = [point_coord, (point_coord + noise*std)[1:]]           # [G,B,N,3]
  normed = (coords - low) / (high - low)
  pe     = interleaved sin/cos embedding, (y,x,z) order            # [G,B,N,384]
  h      = relu(pe @ W1.T + b1)                                    # [G,B,N,512]
  pos    = h @ W2.T + b2                                           # [G,B,N,256]
  query  = label_weight[labels] + pos
  out    = concat([query_pos, query], -1).reshape(G*B, N, 512)

Sharding: data-parallel over the G*B=32 (g,b) pairs, 4 per core, 8 cores.
Each core computes its 4*2048=8192 points' `query` half on device; the
query_pos half is a passthrough assembled on the host.

Device design (v5 = v4 + DMA-queue restructure):
  v4's single-chunk K (8th-order tail collapse, 384 -> 123 rows) kept:
  ONE bf16 matmul per L1 out-block.  New in v5:
  - args (wrapped radians) + poly rows ship as ONE bf16 tile C [128,T];
    Sin runs IN-PLACE on rows 0..97, so C is directly the L1 rhs.
  - label embeddings are gathered on the host and added during PSUM
    evacuation (DVE scalar_tensor_tensor: (q/1024) + lab), dropping the
    two one-hot matmuls: PE runs 8 instructions per tile.
  - DMAs split across both HWDGE queues (SP and ACT) to unclog SP.
"""
import sys
import math

sys.path.insert(0, "/opt/trn_rl_repo")

import numpy as np
import ml_dtypes
from contextlib import ExitStack

import concourse.bass as bass
import concourse.tile as tile
from concourse import bacc, library_config, mybir
from concourse.bass_utils import run_bass_kernel_spmd

# problem constants (hardcoded per contract)
G, B, N, F = 4, 8, 2048, 128
NCORES = 8
BPC = B * G // NCORES          # 4 (g,b) pairs per core
NPTS = BPC * N                 # 8192 points per core
T = 512                        # points per tile
NT = NPTS // T                 # 16 tiles
KKXY, KKZ = 11, 27             # kept frequencies (exact sin) per coord
NKEPT = 4 * KKXY + 2 * KKZ     # 98 kept feature rows
TWO_PI = 2.0 * math.pi
F32 = mybir.dt.float32
BF16 = mybir.dt.bfloat16
FP8 = mybir.dt.float8e4
E4 = ml_dtypes.float8_e4m3
BF = ml_dtypes.bfloat16
DR = mybir.MatmulPerfMode.DoubleRow

_CACHE = {}


def _build_program():
    nc = bacc.Bacc("TRN2", target_bir_lowering=False, debug=False, num_devices=NCORES)

    c_d = nc.dram_tensor("c", [NT, 128, T], BF16, kind="ExternalInput").ap()
    lab_d = nc.dram_tensor("lab", [NT, 128, 2, T], BF16, kind="ExternalInput").ap()
    w1_d = nc.dram_tensor("w1", [128, 512], BF16, kind="ExternalInput").ap()
    w2_d = nc.dram_tensor("w2", [128, 2, 2, 256], FP8, kind="ExternalInput").ap()
    q_d = nc.dram_tensor("q", [128, 2, NPTS], BF16, kind="ExternalOutput").ap()

    with tile.TileContext(nc) as tc, ExitStack() as ctx:
        wpool = ctx.enter_context(tc.tile_pool(name="weights", bufs=1))
        io = ctx.enter_context(tc.tile_pool(name="io", bufs=3))
        work = ctx.enter_context(tc.tile_pool(name="work", bufs=2))
        psum_h = ctx.enter_context(tc.tile_pool(name="ph", bufs=1, space="PSUM"))
        psum_q = ctx.enter_context(tc.tile_pool(name="pq", bufs=2, space="PSUM"))

        w1 = wpool.tile([128, 512], BF16)
        nc.scalar.dma_start(w1[:], w1_d[:])
        w2 = wpool.tile([128, 2, 2, 256], FP8)
        nc.scalar.dma_start(w2[:], w2_d[:])

        # software-pipelined: iteration t does stage-A work (sin, L1, relu)
        # for tile t and stage-B work (L2 + evac) for tile t-1.
        prev = None  # (Hs, lab_t, t-1)
        for t in range(NT + 1):
            if t < NT:
                ct = io.tile([128, T], BF16, tag="c")
                nc.sync.dma_start(ct[:], c_d[t])
                lab_t = io.tile([128, 2, T], BF16, tag="lab")
                nc.sync.dma_start(lab_t[:], lab_d[t])

                # rows 0..97 turn into sin(features) in place; rows 98..127
                # already hold the polynomial tail rows.
                nc.scalar.activation(
                    ct[0:NKEPT, :], ct[0:NKEPT, :], mybir.ActivationFunctionType.Sin
                )

                # L1: h = C @ W1 (x32; b1 + cos constants folded into the
                # ones row), one K=128 bf16 matmul per out-block
                h01 = psum_h.tile([128, 2, T], F32, tag="h01")
                h23 = psum_h.tile([128, 2, T], F32, tag="h23")
                Hs = work.tile([128, 4, T], FP8, tag="hs")
                for half, hp in ((0, h01), (1, h23)):
                    for m2 in range(2):
                        m = half * 2 + m2
                        nc.tensor.matmul(
                            hp[:, m2, :],
                            w1[:, m * 128 : (m + 1) * 128],
                            ct[:],
                            start=True,
                            stop=True,
                        )
                # Hs = 32*relu(...) -> fp8; blocks 0,1 on ACT; 2,3 on DVE
                nc.scalar.activation(
                    Hs[:, 0:2, :], h01[:], mybir.ActivationFunctionType.Relu
                )
                nc.vector.tensor_scalar(
                    Hs[:, 2:4, :], h23[:], 0.0, None, op0=mybir.AluOpType.max
                )

            if prev is not None:
                Hp, labp, tp = prev
                rhs2 = (Hp[:, 0:2, :], Hp[:, 2:4, :])
                qp = psum_q.tile([128, 2, T], F32, tag="qp")
                for mp in range(2):
                    for d in range(2):
                        nc.tensor.matmul(
                            qp[:, mp, :],
                            w2[:, d, :, mp * 128 : (mp + 1) * 128],
                            rhs2[d],
                            start=(d == 0),
                            stop=(d == 1),
                            perf_mode=DR,
                        )
                # qs = q/1024 + lab_emb, evacuated to bf16 in one DVE inst
                qs = work.tile([128, 2, T], BF16, tag="qs")
                nc.vector.scalar_tensor_tensor(
                    qs[:],
                    qp[:],
                    1.0 / 1024.0,
                    labp[:],
                    op0=mybir.AluOpType.mult,
                    op1=mybir.AluOpType.add,
                )
                nc.sync.dma_start(q_d[:, 0, tp * T : (tp + 1) * T], qs[:, 0, :])
                nc.scalar.dma_start(q_d[:, 1, tp * T : (tp + 1) * T], qs[:, 1, :])

            if t < NT:
                prev = (Hs, lab_t, t)

    nc.compile()
    return nc


def _row_plan():
    kks = (KKXY, KKXY, KKZ)
    starts = []
    off = 0
    for c in range(3):
        starts.append(off)
        off += 2 * kks[c]
    return kks, starts, off


def _host_prep(point_coord, labels, pc_range, noise, label_weight, W1, b1, W2, b2):
    """Build the per-core input maps (host-side sharding + weight prep)."""
    pc32 = np.asarray(point_coord, np.float32)
    lab = np.asarray(labels)
    noi = np.asarray(noise, np.float32)
    rng = np.asarray(pc_range, np.float32)

    small = (lab == 0) | (lab >= 6)
    std = np.where(small, 2.0, 4.0).astype(np.float32)            # [B,N]
    coords = pc32[None] + noi * std[None, :, :, None]             # [G,B,N,3]
    coords[0] = pc32                                              # group 0 originals
    low, high = rng[:3], rng[3:]
    pcs = (coords - low) / (high - low) * np.float32(TWO_PI)      # [G,B,N,3]
    pcs = pcs[..., [1, 0, 2]]   # reference concatenates pe in (y,x,z) order

    W1f = np.asarray(W1, np.float32)    # [512, 384]
    b1f = np.asarray(b1, np.float32)
    kk64 = np.arange(64, dtype=np.float64)
    s64 = 10000.0 ** (-kk64 / 64.0)

    def fi(c, k, cos):
        return c * 128 + 2 * k + (1 if cos else 0)

    kks, starts, poly_base = _row_plan()

    # --- W1 single chunk (x32): kept rows + folded polynomial tail ---
    w1t = np.zeros((128, 512), np.float32)
    const_acc = b1f.astype(np.float64).copy()
    for c in range(3):
        kk = kks[c]
        st = starts[c]
        sin_idx = [fi(c, k, False) for k in range(kk)]
        cos_idx = [fi(c, k, True) for k in range(kk)]
        w1t[st : st + kk] = 32.0 * W1f[:, sin_idx].T
        w1t[st + kk : st + 2 * kk] = 32.0 * W1f[:, cos_idx].T
        s_t = s64[kk:]
        sc = W1f[:, [fi(c, k, False) for k in range(kk, 64)]].astype(np.float64)
        cc = W1f[:, [fi(c, k, True) for k in range(kk, 64)]].astype(np.float64)
        for p in range(1, 9):
            fac = math.factorial(p)
            if p % 2 == 1:
                sign = -1.0 if (p - 1) // 2 % 2 else 1.0
                coef = sc @ (sign * s_t**p / fac)
            else:
                sign = -1.0 if (p // 2) % 2 else 1.0
                coef = cc @ (sign * s_t**p / fac)
            w1t[poly_base + 8 * c + (p - 1)] = 32.0 * coef
        const_acc += cc.sum(axis=1)
    w1t[poly_base + 24] = 32.0 * const_acc
    w1t = w1t.astype(BF)

    # --- W2 (x32) -> fp8 hi, DoubleRow pairs (k0,k1),(k2,k3) ---
    A2 = (32.0 * np.asarray(W2, np.float32).T).astype(np.float32)     # [512,256]
    w2t = np.empty((128, 2, 2, 256), E4)
    for d in range(2):
        for i in range(2):
            k = 2 * d + i
            w2t[:, d, i] = A2[k * 128 : (k + 1) * 128].astype(E4)

    # --- label embedding table (+b2), gathered on host ---
    lwbt = (
        np.asarray(label_weight, np.float32) + np.asarray(b2, np.float32)
    )                                                    # [10, 256]
    lab_full = lwbt[np.asarray(lab, np.int64)]           # [B, N, 256]

    shared = {"w1": w1t, "w2": w2t}

    in_maps = []
    for core in range(NCORES):
        g = core // 2
        b0 = 4 * (core % 2)
        pcc = pcs[g, b0 : b0 + 4].reshape(NPTS, 3).T.astype(np.float64)  # [3,NPTS]

        # combined C tile: wrapped radians (bf16) for kept rows + poly rows
        carr = np.zeros((128, NPTS), np.float32)
        for c in range(3):
            kk = kks[c]
            st = starts[c]
            sv = s64[:kk]
            a = sv[:, None] * pcc[c][None]
            carr[st : st + kk] = (np.mod(a + np.pi, TWO_PI) - np.pi)
            a = a + np.pi / 2
            carr[st + kk : st + 2 * kk] = (np.mod(a + np.pi, TWO_PI) - np.pi)
            v = np.ones_like(pcc[c])
            for p in range(8):
                v = v * pcc[c]
                carr[poly_base + 8 * c + p] = v
        carr[poly_base + 24] = 1.0
        carr = np.ascontiguousarray(
            carr.astype(BF).reshape(128, NT, T).transpose(1, 0, 2)
        )                                               # [NT,128,T]

        labc = lab_full[b0 : b0 + 4].reshape(NPTS, 256).T   # [256, NPTS]
        labc = labc.reshape(2, 128, NPTS).transpose(1, 0, 2)  # [128,2,NPTS]
        labc = np.ascontiguousarray(
            labc.astype(BF).reshape(128, 2, NT, T).transpose(2, 0, 1, 3)
        )                                               # [NT,128,2,T]
        in_maps.append({"c": carr, "lab": labc, **shared})
    return in_maps


def _get_nc():
    if "nc" not in _CACHE:
        _CACHE["nc"] = _build_program()
    return _CACHE["nc"]


def _run_device(in_maps, trace=False, **kw):
    nc = _get_nc()
    return run_bass_kernel_spmd(nc, in_maps, list(range(NCORES)), trace=trace, **kw)


def kernel(point_coord, labels, pc_range, noise, query_pos, label_weight, W1, b1, W2, b2):
    in_maps = _host_prep(
        point_coord, labels, pc_range, noise, label_weight, W1, b1, W2, b2
    )
    res = _run_device(in_maps)

    qp = np.asarray(query_pos, np.float32)
    out = np.empty((G * B, N, 4 * F), np.float32)
    out[:, :, : 2 * F] = qp.reshape(G * B, N, 2 * F)
    for core in range(NCORES):
        q = np.asarray(res.results[core]["q"], np.float32)  # [128,2,NPTS]
        q = q.transpose(1, 0, 2).reshape(2 * F, BPC, N).transpose(1, 2, 0)
        out[4 * core : 4 * core + 4, :, 2 * F :] = q        # [4, N, 256]
    return out
